# revision 1
# baseline (speedup 1.0000x reference)
"""Trainium2 Bass kernel for nn_JResCOPAttn (B=1, L=1024, D=128).

Reference computation:
    a   = x @ Wl.T + bl                        # [L, D]
    tm  = (a[:,None,:] * a[None,:,:]) @ Wlo.T + blo    # [L, L, D]  (never materialized!)
    tm *= (mask != 0)
    tx  = x @ Wl2.T + bl2                      # [L, D]
    y   = x + einsum('cad,ad->cd', tm, tx)
    out = LayerNorm(y) * gamma + beta

Algebraic restructuring used here (per output row c):
    y1[c,d] = sum_e act[c,e] * WloT[e,d] * S_c[e,d]  +  blo[d] * Z[c,d]
    S_c[e,d] = sum_a (mask[c,a]*act[a,e]) * tx[a,d]      (8 accumulating matmuls)
    Z[c,d]   = sum_a mask[c,a] * tx[a,d]                 (one batch of matmuls)
This avoids materializing the 536MB tm tensor entirely.

Sharding: rows c are split across the 8 NeuronCores (128 rows each); x is
replicated so each core computes act/tx for all 1024 source rows locally.
"""

import os
import sys

for _p in ("/opt/trn_rl_repo", "/root/.axon_site/_ro/trn_rl_repo"):
    if os.path.isdir(_p) and _p not in sys.path:
        sys.path.insert(0, _p)

import numpy as np

import concourse.bass as bass
import concourse.tile as tile
from concourse import bacc, mybir
from concourse.bass_utils import run_bass_kernel_spmd
from concourse.masks import make_identity

B, L, D = 1, 1024, 128
NCORES = 8
CB = L // NCORES          # c-rows per core = 128
T = L // 128              # a-tiles = 8
EPS = 1e-5
FP = mybir.dt.float32

# how many of the 8 per-c mask-apply ops run on DVE (rest on ScalarE/ACT)
N_DVE_MASK = 5
QUAD = 4                  # c's sharing one PSUM bank / one G multiply


def build_nc():
    nc = bacc.Bacc("TRN2", target_bir_lowering=False)

    # ---- I/O ----
    xT   = nc.dram_tensor("xT",   [128, L], FP, kind="ExternalInput")    # x^T (feature-major)
    xTb  = nc.dram_tensor("xTb",  [128, CB], FP, kind="ExternalInput")   # this core's block of xT cols
    xrow = nc.dram_tensor("xrow", [CB, D], FP, kind="ExternalInput")     # this core's x rows (residual)
    mT   = nc.dram_tensor("mT",   [128, T, CB], FP, kind="ExternalInput")  # mT[p,t,c] = mask[c0+c, t*128+p]
    WlT  = nc.dram_tensor("WlT",  [128, 128], FP, kind="ExternalInput")  # Wl.T
    Wl2T = nc.dram_tensor("Wl2T", [128, 128], FP, kind="ExternalInput")  # Wl2.T
    Wlo4 = nc.dram_tensor("Wlo4", [128, QUAD, 128], FP, kind="ExternalInput")  # Wlo.T replicated QUADx
    bl   = nc.dram_tensor("bl",   [128, 1], FP, kind="ExternalInput")
    bl2  = nc.dram_tensor("bl2",  [128, 1], FP, kind="ExternalInput")
    blo  = nc.dram_tensor("blo",  [128, 1], FP, kind="ExternalInput")
    gam  = nc.dram_tensor("gam",  [CB, D], FP, kind="ExternalInput")     # gamma broadcast to rows
    bet  = nc.dram_tensor("bet",  [CB, D], FP, kind="ExternalInput")
    out  = nc.dram_tensor("out",  [CB, D], FP, kind="ExternalOutput")

    Ident = mybir.ActivationFunctionType.Identity
    Sqrt = mybir.ActivationFunctionType.Sqrt
    mult = mybir.AluOpType.mult

    with tile.TileContext(nc) as tc:
        with (
            tc.tile_pool(name="singles", bufs=1) as singles,
            tc.tile_pool(name="trps", bufs=2, space="PSUM") as trps,
            tc.tile_pool(name="setps", bufs=2, space="PSUM") as setps,
            tc.tile_pool(name="ma", bufs=3) as ma_pool,
            tc.tile_pool(name="g", bufs=2) as g_pool,
            tc.tile_pool(name="s4", bufs=2, space="PSUM") as s4_pool,
            tc.tile_pool(name="y1tp", bufs=1, space="PSUM") as y1t_pool,
        ):
            # ---- load constants / inputs ----
            sb_xT = singles.tile([128, L], FP)
            nc.sync.dma_start(sb_xT, xT[:, :])
            sb_xTb = singles.tile([128, CB], FP)
            nc.sync.dma_start(sb_xTb, xTb[:, :])
            sb_xrow = singles.tile([CB, D], FP)
            nc.sync.dma_start(sb_xrow, xrow[:, :])
            sb_mT = singles.tile([128, T, CB], FP)
            nc.sync.dma_start(sb_mT, mT[:, :, :])
            sb_WlT = singles.tile([128, 128], FP)
            nc.sync.dma_start(sb_WlT, WlT[:, :])
            sb_Wl2T = singles.tile([128, 128], FP)
            nc.sync.dma_start(sb_Wl2T, Wl2T[:, :])
            sb_Wlo4 = singles.tile([128, QUAD, 128], FP)
            nc.sync.dma_start(sb_Wlo4, Wlo4[:, :, :])
            sb_bl = singles.tile([128, 1], FP)
            nc.sync.dma_start(sb_bl, bl[:, :])
            sb_bl2 = singles.tile([128, 1], FP)
            nc.sync.dma_start(sb_bl2, bl2[:, :])
            sb_blo = singles.tile([128, 1], FP)
            nc.sync.dma_start(sb_blo, blo[:, :])
            sb_gam = singles.tile([CB, D], FP)
            nc.sync.dma_start(sb_gam, gam[:, :])
            sb_bet = singles.tile([CB, D], FP)
            nc.sync.dma_start(sb_bet, bet[:, :])

            ident = singles.tile([128, 128], FP)
            make_identity(nc, ident)
            sb_eps = singles.tile([CB, 1], FP)
            nc.vector.memset(sb_eps, EPS)

            # ---- actT / txT = W @ xT + bias  (feature-major activations) ----
            actT = singles.tile([128, L], FP)
            txT = singles.tile([128, L], FP)
            for h in range(2):
                sl = slice(h * 512, (h + 1) * 512)
                ps_a = setps.tile([128, 512], FP, tag="set_mm")
                nc.tensor.matmul(ps_a, sb_WlT, sb_xT[:, sl], start=True, stop=True)
                nc.scalar.activation(actT[:, sl], ps_a, Ident, bias=sb_bl, scale=1.0)
                ps_t = setps.tile([128, 512], FP, tag="set_mm")
                nc.tensor.matmul(ps_t, sb_Wl2T, sb_xT[:, sl], start=True, stop=True)
                nc.scalar.activation(txT[:, sl], ps_t, Ident, bias=sb_bl2, scale=1.0)

            # actT restricted to this core's c-block (for the reduce matmuls)
            actTb = singles.tile([128, CB], FP)
            ps_b = setps.tile([128, 512], FP, tag="set_mm")
            nc.tensor.matmul(ps_b[:, :CB], sb_WlT, sb_xTb, start=True, stop=True)
            nc.scalar.activation(actTb, ps_b[:, :CB], Ident, bias=sb_bl, scale=1.0)

            # ---- natural-layout act / tx tiles via PE transpose ----
            act_nat = singles.tile([128, T, 128], FP)
            tx_nat = singles.tile([128, T, 128], FP)
            for t in range(T):
                sl = slice(t * 128, (t + 1) * 128)
                p1 = trps.tile([128, 128], FP, tag="tr")
                nc.tensor.transpose(p1, actT[:, sl], ident)
                nc.vector.tensor_copy(act_nat[:, t, :], p1)
                p2 = trps.tile([128, 128], FP, tag="tr")
                nc.tensor.transpose(p2, txT[:, sl], ident)
                nc.vector.tensor_copy(tx_nat[:, t, :], p2)

            # ---- ZT[d,c] = sum_a tx[a,d] * mask[c,a];  bloZT = blo * ZT ----
            zt_ps = setps.tile([128, 512], FP, tag="set_mm")
            for t in range(T):
                nc.tensor.matmul(
                    zt_ps[:, :CB], tx_nat[:, t, :], sb_mT[:, t, :],
                    start=(t == 0), stop=(t == T - 1),
                )
            bloZT = singles.tile([128, CB], FP)
            nc.vector.tensor_scalar_mul(bloZT, zt_ps[:, :CB], sb_blo)

            # ---- main loop over this core's 128 output rows ----
            y1t_ps = y1t_pool.tile([128, CB], FP)  # Y1^T columns, [d, c]
            for cq in range(CB // QUAD):
                s4 = s4_pool.tile([128, QUAD, 128], FP)
                for j in range(QUAD):
                    c = cq * QUAD + j
                    ma = ma_pool.tile([128, T, 128], FP, tag="ma")
                    for t in range(T):
                        if t < N_DVE_MASK:
                            nc.vector.tensor_scalar_mul(
                                ma[:, t, :], act_nat[:, t, :], sb_mT[:, t, c:c + 1]
                            )
                        else:
                            nc.scalar.mul(
                                ma[:, t, :], act_nat[:, t, :], sb_mT[:, t, c:c + 1]
                            )
                    for t in range(T):
                        nc.tensor.matmul(
                            s4[:, j, :], ma[:, t, :], tx_nat[:, t, :],
                            start=(t == 0), stop=(t == T - 1),
                        )
                g4 = g_pool.tile([128, QUAD, 128], FP, tag="g4")
                nc.vector.tensor_mul(g4, s4, sb_Wlo4)
                for j in range(QUAD):
                    c = cq * QUAD + j
                    nc.tensor.matmul(
                        y1t_ps[:, c:c + 1], g4[:, j, :], actTb[:, c:c + 1],
                        start=True, stop=True,
                    )

            # ---- combine, transpose back, residual, LayerNorm ----
            yt_sb = singles.tile([128, CB], FP)
            nc.vector.tensor_add(yt_sb, y1t_ps, bloZT)           # [d, c]
            y_ps = trps.tile([128, 128], FP, tag="tr")
            nc.tensor.transpose(y_ps, yt_sb, ident)              # [c, d]
            y_sb = singles.tile([CB, D], FP)
            nc.vector.tensor_add(y_sb, y_ps, sb_xrow)            # + x residual

            stats = singles.tile([CB, nc.vector.BN_STATS_DIM], FP)
            nc.vector.bn_stats(stats, y_sb)
            mv = singles.tile([CB, 2], FP)
            nc.vector.bn_aggr(mv, stats)
            nc.vector.tensor_scalar_sub(y_sb, y_sb, mv[:, 0:1])  # y - mean
            sd = singles.tile([CB, 1], FP)
            nc.scalar.activation(sd, mv[:, 1:2], Sqrt, bias=sb_eps, scale=1.0)
            rstd = singles.tile([CB, 1], FP)
            nc.vector.reciprocal(rstd, sd)
            nc.vector.tensor_scalar_mul(y_sb, y_sb, rstd)
            nc.vector.tensor_mul(y_sb, y_sb, sb_gam)
            nc.vector.tensor_add(y_sb, y_sb, sb_bet)

            nc.sync.dma_start(out[:, :], y_sb)

    return nc


_NC_CACHE = None


def _get_nc():
    global _NC_CACHE
    if _NC_CACHE is None:
        _NC_CACHE = build_nc()
        _NC_CACHE.finalize()
    return _NC_CACHE


def _prepare_in_maps(x, mask, Wl, bl, Wlo, blo, Wl2, bl2, gamma, beta):
    f32 = np.float32
    x0 = np.ascontiguousarray(np.asarray(x, f32)[0])          # [L, D]
    m = np.asarray(mask)[0].astype(f32)                       # [L, L] (c, a)
    xT = np.ascontiguousarray(x0.T)                           # [128, L]
    WlT = np.ascontiguousarray(np.asarray(Wl, f32).T)
    Wl2T = np.ascontiguousarray(np.asarray(Wl2, f32).T)
    WloT = np.ascontiguousarray(np.asarray(Wlo, f32).T)       # [e, d]
    Wlo4 = np.ascontiguousarray(
        np.broadcast_to(WloT[:, None, :], (128, QUAD, 128)).astype(f32)
    )
    bl_c = np.asarray(bl, f32).reshape(128, 1)
    bl2_c = np.asarray(bl2, f32).reshape(128, 1)
    blo_c = np.asarray(blo, f32).reshape(128, 1)
    gam_b = np.ascontiguousarray(np.broadcast_to(np.asarray(gamma, f32), (CB, D)))
    bet_b = np.ascontiguousarray(np.broadcast_to(np.asarray(beta, f32), (CB, D)))

    in_maps = []
    for k in range(NCORES):
        blk = slice(k * CB, (k + 1) * CB)
        mTk = m[blk, :].T.reshape(T, 128, CB).transpose(1, 0, 2)  # [p, t, c]
        in_maps.append({
            "xT": xT,
            "xTb": np.ascontiguousarray(xT[:, blk]),
            "xrow": np.ascontiguousarray(x0[blk]),
            "mT": np.ascontiguousarray(mTk),
            "WlT": WlT,
            "Wl2T": Wl2T,
            "Wlo4": Wlo4,
            "bl": bl_c,
            "bl2": bl2_c,
            "blo": blo_c,
            "gam": gam_b,
            "bet": bet_b,
        })
    return in_maps


def kernel(x, mask, Wl, bl, Wlo, blo, Wl2, bl2, gamma, beta):
    in_maps = _prepare_in_maps(x, mask, Wl, bl, Wlo, blo, Wl2, bl2, gamma, beta)
    res = run_bass_kernel_spmd(_get_nc(), in_maps, core_ids=list(range(NCORES)))
    y = np.concatenate([res.results[k]["out"] for k in range(NCORES)], axis=0)
    return y.reshape(B, L, D).astype(np.float32)



# revision 4
# speedup vs baseline: 1.6489x; 1.6489x over previous
"""Trainium2 Bass kernel for nn_JResCOPAttn (B=1, L=1024, D=128).

Reference computation:
    a   = x @ Wl.T + bl                                # [L, D]
    tm  = (a[:,None,:] * a[None,:,:]) @ Wlo.T + blo    # [L, L, D]  (never materialized!)
    tm *= (mask != 0)
    tx  = x @ Wl2.T + bl2                              # [L, D]
    y   = x + einsum('cad,ad->cd', tm, tx)
    out = LayerNorm(y) * gamma + beta

Algebraic restructuring (e indexes the D channels of `a`):
    y1[c,d] = sum_e a[c,e] * T'[c,e,d]  +  blo[d] * Z[c,d]
    T'[c,e,d] = sum_a mask[c,a] * (a[a,e] * tx[a,d] * Wlo[d,e])
    Z[c,d]    = sum_a mask[c,a] * tx[a,d]

Sharding: the e axis (128 channels) is split across the 8 cores (16 each).
Every core computes its 16-channel partial y1 for ALL 1024 output rows:
    U'[a, j, d] = a[a, e0+j] * tx[a,d] * Wlo[d, e0+j]   (bf16, DVE fused op)
    T' = mask @ U'                                      (bf16 matmuls, N=512)
    partial[c,d] = sum_j a[c, e0+j] * T'[c,j,d]         (ACT scale + DVE tree)
then a ReduceScatter over HBM sums the 8 partials and hands each core its own
128-row c-shard, where the Z term, residual and LayerNorm are applied.
The per-core e-chunk enters only through input tensors (WlTc / WloBc), so a
single compiled program serves all 8 cores.
"""

import os
import sys

for _p in ("/opt/trn_rl_repo", "/root/.axon_site/_ro/trn_rl_repo"):
    if os.path.isdir(_p) and _p not in sys.path:
        sys.path.insert(0, _p)

import numpy as np
import ml_dtypes

import concourse.tile as tile
from concourse import bacc, mybir
from concourse.bass_utils import run_bass_kernel_spmd

B, L, D = 1, 1024, 128
NCORES = 8
CB = L // NCORES          # c-rows per core shard = 128
T = L // 128              # a-tiles / c-tiles = 8
EC = D // NCORES          # e-channels per core = 16
EPS = 1e-5
FP = mybir.dt.float32
BF = mybir.dt.bfloat16

NSPLIT = 4                # 512-wide n-chunks per (ct, t) matmul group


def build_nc():
    nc = bacc.Bacc("TRN2", target_bir_lowering=False, num_devices=NCORES)

    # ---- I/O (per-core tensors; e-chunk/c-shard baked into the data) ----
    xT    = nc.dram_tensor("xT",    [128, L], FP, kind="ExternalInput")      # x^T
    xrow  = nc.dram_tensor("xrow",  [CB, D], FP, kind="ExternalInput")       # own c-shard of x
    mT    = nc.dram_tensor("mT",    [128, T, L], BF, kind="ExternalInput")   # mT[p,t,c] = mask[c, t*128+p]
    mTc   = nc.dram_tensor("mTc",   [128, T, CB], BF, kind="ExternalInput")  # own-shard mask cols
    WlTc  = nc.dram_tensor("WlTc",  [128, EC], FP, kind="ExternalInput")     # Wl.T[:, e0:e0+EC]
    Wl2T  = nc.dram_tensor("Wl2T",  [128, 128], FP, kind="ExternalInput")    # Wl2.T
    WloBc = nc.dram_tensor("WloBc", [128, EC, 128], BF, kind="ExternalInput")  # Wlo[d, e0+j], bcast parts
    blcB  = nc.dram_tensor("blcB",  [128, EC], FP, kind="ExternalInput")     # bl[e0:e0+EC] bcast parts
    bl2B  = nc.dram_tensor("bl2B",  [128, 128], FP, kind="ExternalInput")    # bl2 bcast parts
    bloB  = nc.dram_tensor("bloB",  [CB, D], FP, kind="ExternalInput")       # blo bcast parts
    gam   = nc.dram_tensor("gam",   [CB, D], FP, kind="ExternalInput")
    bet   = nc.dram_tensor("bet",   [CB, D], FP, kind="ExternalInput")
    out   = nc.dram_tensor("out",   [CB, D], FP, kind="ExternalOutput")

    Sqrt = mybir.ActivationFunctionType.Sqrt
    mult = mybir.AluOpType.mult
    add = mybir.AluOpType.add

    with tile.TileContext(nc) as tc:
        with (
            tc.tile_pool(name="singles", bufs=1) as singles,
            tc.tile_pool(name="mm", bufs=2, space="PSUM") as mmps,
            tc.tile_pool(name="v", bufs=2) as vpool,
            tc.tile_pool(name="tr", bufs=2) as trpool,
            tc.tile_pool(name="dram", bufs=1, space="DRAM") as dram,
        ):
            # ---- load inputs ----
            sb_xT = singles.tile([128, L], FP)
            nc.sync.dma_start(sb_xT, xT[:, :])
            sb_WlTc = singles.tile([128, EC], FP)
            nc.sync.dma_start(sb_WlTc, WlTc[:, :])
            sb_Wl2T = singles.tile([128, 128], FP)
            nc.sync.dma_start(sb_Wl2T, Wl2T[:, :])
            sb_WloBc = singles.tile([128, EC, 128], BF)
            nc.sync.dma_start(sb_WloBc, WloBc[:, :, :])
            sb_blcB = singles.tile([128, EC], FP)
            nc.sync.dma_start(sb_blcB, blcB[:, :])
            sb_bl2B = singles.tile([128, 128], FP)
            nc.sync.dma_start(sb_bl2B, bl2B[:, :])
            sb_mT = singles.tile([128, T, L], BF)
            nc.sync.dma_start(sb_mT, mT[:, :, :])
            sb_mTc = singles.tile([128, T, CB], BF)
            nc.sync.dma_start(sb_mTc, mTc[:, :, :])
            sb_bloB = singles.tile([CB, D], FP)
            nc.sync.dma_start(sb_bloB, bloB[:, :])
            sb_xrow = singles.tile([CB, D], FP)
            nc.sync.dma_start(sb_xrow, xrow[:, :])
            sb_gam = singles.tile([CB, D], FP)
            nc.sync.dma_start(sb_gam, gam[:, :])
            sb_bet = singles.tile([CB, D], FP)
            nc.sync.dma_start(sb_bet, bet[:, :])
            sb_eps = singles.tile([CB, 1], FP)
            nc.vector.memset(sb_eps, EPS)

            # ---- activations in natural layout (rows on partitions), bf16 ----
            # tx_nat[p, t, d] = tx[t*128+p, d];  a_sel[p, t, j] = a[t*128+p, e0+j]
            tx_nat = singles.tile([128, T, 128], BF)
            a_sel = singles.tile([128, T, EC], FP)
            for h in range(2):
                ps = mmps.tile([128, NSPLIT, 512], FP, tag="mm")
                for q in range(4):
                    t = h * 4 + q
                    sl = slice(t * 128, (t + 1) * 128)
                    nc.tensor.matmul(ps[:, q, 0:128], sb_xT[:, sl], sb_Wl2T,
                                     start=True, stop=True)
                    nc.tensor.matmul(ps[:, q, 128:128 + EC], sb_xT[:, sl], sb_WlTc,
                                     start=True, stop=True)
                for q in range(4):
                    t = h * 4 + q
                    nc.vector.tensor_add(tx_nat[:, t, :], ps[:, q, 0:128], sb_bl2B)
                    nc.vector.tensor_add(a_sel[:, t, :], ps[:, q, 128:128 + EC],
                                         sb_blcB)

            # ---- Z term for own shard: Z[c,d] = sum_a mask[c,a] tx[a,d] ----
            z_ps = mmps.tile([128, NSPLIT, 512], FP, tag="mm")
            for t in range(T):
                nc.tensor.matmul(z_ps[:, 0, 0:CB], sb_mTc[:, t, :], tx_nat[:, t, :],
                                 start=(t == 0), stop=(t == T - 1))
            # base = x_shard + blo * Z   (everything not needing the collective)
            base = singles.tile([CB, D], FP)
            nc.vector.scalar_tensor_tensor(base, z_ps[:, 0, 0:CB], 1.0, sb_bloB,
                                           op0=mult, op1=mult)
            nc.vector.tensor_add(base, base, sb_xrow)

            # ---- U'[a, j, d] = a[a,e0+j] * tx[a,d] * Wlo[d,e0+j]  (bf16) ----
            up = singles.tile([128, T, EC * 128], BF)
            for t in range(T):
                for j in range(EC):
                    nc.vector.scalar_tensor_tensor(
                        up[:, t, j * 128:(j + 1) * 128],
                        tx_nat[:, t, :],
                        a_sel[:, t, j:j + 1],
                        sb_WloBc[:, j, :],
                        op0=mult, op1=mult,
                    )

            # ---- main loop over output c-tiles ----
            part_dram = dram.tile([L, D], FP)
            for ct in range(T):
                ps = mmps.tile([128, NSPLIT, 512], FP, tag="mm")
                for t in range(T):
                    lhsT = sb_mT[:, t, ct * 128:(ct + 1) * 128]
                    for n in range(NSPLIT):
                        nc.tensor.matmul(
                            ps[:, n, :], lhsT, up[:, t, n * 512:(n + 1) * 512],
                            start=(t == 0), stop=(t == T - 1),
                        )
                # V[c, j, d] = a[c, e0+j] * T'[c, j, d]   (ACT, PSUM->SBUF, bf16)
                v = vpool.tile([128, EC, 128], BF, tag="v")
                for j in range(EC):
                    nc.scalar.mul(v[:, j, :],
                                  ps[:, j // 4, (j % 4) * 128:(j % 4 + 1) * 128],
                                  a_sel[:, ct, j:j + 1])
                # tree-reduce over j -> partial[c, d]
                t8 = trpool.tile([128, 8, 128], BF, tag="t8")
                nc.vector.tensor_add(t8, v[:, 0:8, :], v[:, 8:16, :])
                t4 = trpool.tile([128, 4, 128], BF, tag="t4")
                nc.vector.tensor_add(t4, t8[:, 0:4, :], t8[:, 4:8, :])
                t2 = trpool.tile([128, 2, 128], BF, tag="t2")
                nc.vector.tensor_add(t2, t4[:, 0:2, :], t4[:, 2:4, :])
                p1 = trpool.tile([128, 128], FP, tag="p1")
                nc.vector.tensor_add(p1, t2[:, 0, :], t2[:, 1, :])
                nc.sync.dma_start(part_dram[ct * 128:(ct + 1) * 128, :], p1)

            # ---- cross-core reduce-scatter over e-partials ----
            red_dram = dram.tile([CB, D], FP)
            nc.gpsimd.collective_compute(
                "ReduceScatter",
                add,
                replica_groups=[list(range(NCORES))],
                ins=[part_dram[:, :].opt()],
                outs=[red_dram[:, :].opt()],
            )
            red_sb = singles.tile([CB, D], FP)
            nc.sync.dma_start(red_sb, red_dram[:, :])

            # ---- y = base + reduced partials ; LayerNorm ----
            y_sb = singles.tile([CB, D], FP)
            nc.vector.tensor_add(y_sb, red_sb, base)

            stats = singles.tile([CB, nc.vector.BN_STATS_DIM], FP)
            nc.vector.bn_stats(stats, y_sb)
            mv = singles.tile([CB, 2], FP)
            nc.vector.bn_aggr(mv, stats)
            nc.vector.tensor_scalar_sub(y_sb, y_sb, mv[:, 0:1])
            sd = singles.tile([CB, 1], FP)
            nc.scalar.activation(sd, mv[:, 1:2], Sqrt, bias=sb_eps, scale=1.0)
            rstd = singles.tile([CB, 1], FP)
            nc.vector.reciprocal(rstd, sd)
            nc.vector.tensor_scalar_mul(y_sb, y_sb, rstd)
            nc.vector.tensor_mul(y_sb, y_sb, sb_gam)
            nc.vector.tensor_add(y_sb, y_sb, sb_bet)

            nc.sync.dma_start(out[:, :], y_sb)

    return nc


_NC_CACHE = None


def _get_nc():
    global _NC_CACHE
    if _NC_CACHE is None:
        _NC_CACHE = build_nc()
        _NC_CACHE.finalize()
    return _NC_CACHE


def _prepare_in_maps(x, mask, Wl, bl, Wlo, blo, Wl2, bl2, gamma, beta):
    f32 = np.float32
    bf16 = ml_dtypes.bfloat16
    x0 = np.ascontiguousarray(np.asarray(x, f32)[0])          # [L, D]
    xT = np.ascontiguousarray(x0.T)                           # [128, L]
    m = np.asarray(mask)[0].astype(bf16)                      # [L(c), L(a)]
    # mT[p, t, c] = mask[c, t*128 + p]
    mT_full = np.ascontiguousarray(
        m.T.reshape(T, 128, L).transpose(1, 0, 2))            # [128, T, L]
    WlT = np.ascontiguousarray(np.asarray(Wl, f32).T)         # [in, e]
    Wl2T = np.ascontiguousarray(np.asarray(Wl2, f32).T)
    WloT = np.asarray(Wlo, f32).T                             # [e, d]
    bl_ = np.asarray(bl, f32)
    bl2B = np.ascontiguousarray(np.broadcast_to(np.asarray(bl2, f32), (128, 128)))
    bloB = np.ascontiguousarray(np.broadcast_to(np.asarray(blo, f32), (CB, D)))
    gam_b = np.ascontiguousarray(np.broadcast_to(np.asarray(gamma, f32), (CB, D)))
    bet_b = np.ascontiguousarray(np.broadcast_to(np.asarray(beta, f32), (CB, D)))

    in_maps = []
    for k in range(NCORES):
        blk = slice(k * CB, (k + 1) * CB)
        e0 = k * EC
        WloBc = np.ascontiguousarray(
            np.broadcast_to(WloT[e0:e0 + EC][None, :, :].astype(bf16),
                            (128, EC, 128)))
        in_maps.append({
            "xT": xT,
            "xrow": np.ascontiguousarray(x0[blk]),
            "mT": mT_full,
            "mTc": np.ascontiguousarray(mT_full[:, :, blk]),
            "WlTc": np.ascontiguousarray(WlT[:, e0:e0 + EC]),
            "Wl2T": Wl2T,
            "WloBc": WloBc,
            "blcB": np.ascontiguousarray(
                np.broadcast_to(bl_[e0:e0 + EC], (128, EC))),
            "bl2B": bl2B,
            "bloB": bloB,
            "gam": gam_b,
            "bet": bet_b,
        })
    return in_maps


def kernel(x, mask, Wl, bl, Wlo, blo, Wl2, bl2, gamma, beta):
    in_maps = _prepare_in_maps(x, mask, Wl, bl, Wlo, blo, Wl2, bl2, gamma, beta)
    res = run_bass_kernel_spmd(_get_nc(), in_maps, core_ids=list(range(NCORES)))
    y = np.concatenate([res.results[k]["out"] for k in range(NCORES)], axis=0)
    return y.reshape(B, L, D).astype(np.float32)


# revision 11
# speedup vs baseline: 1.9016x; 1.1533x over previous
"""Trainium2 Bass kernel for nn_JResCOPAttn (B=1, L=1024, D=128).

Reference computation:
    a   = x @ Wl.T + bl                                # [L, D]
    tm  = (a[:,None,:] * a[None,:,:]) @ Wlo.T + blo    # [L, L, D]  (never materialized!)
    tm *= (mask != 0)
    tx  = x @ Wl2.T + bl2                              # [L, D]
    y   = x + einsum('cad,ad->cd', tm, tx)
    out = LayerNorm(y) * gamma + beta

Algebraic restructuring (e indexes the D channels of `a`):
    y1[c,d] = sum_e a[c,e] * T'[c,e,d]  +  blo[d] * Z[c,d]
    T'[c,e,d] = sum_a mask[c,a] * (a[a,e] * tx[a,d] * Wlo[d,e])
    Z[c,d]    = sum_a mask[c,a] * tx[a,d]

Sharding: the e axis (128 channels) is split across the 8 cores (16 each).
Every core computes its 16-channel partial y1 for ALL 1024 output rows:
    U'[a, j, d] = a[a, e0+j] * tx[a,d] * Wlo[d, e0+j]   (bf16, DVE fused op)
    T' = mask @ U'                                      (bf16 matmuls, N=512)
    partial[c,d] = sum_j a[c, e0+j] * T'[c,j,d]         (ACT scale + DVE tree)
then a ReduceScatter over HBM sums the 8 partials and hands each core its own
128-row c-shard, where the Z term, residual and LayerNorm are applied.
The per-core e-chunk enters only through input tensors (WlTc / WloBc), so a
single compiled program serves all 8 cores.
"""

import os
import sys

for _p in ("/opt/trn_rl_repo", "/root/.axon_site/_ro/trn_rl_repo"):
    if os.path.isdir(_p) and _p not in sys.path:
        sys.path.insert(0, _p)

import numpy as np
import ml_dtypes

import concourse.tile as tile
from concourse import bacc, mybir
from concourse.bass_utils import run_bass_kernel_spmd

B, L, D = 1, 1024, 128
NCORES = 8
CB = L // NCORES          # c-rows per core shard = 128
T = L // 128              # a-tiles / c-tiles = 8
EC = D // NCORES          # e-channels per core = 16
EPS = 1e-5
FP = mybir.dt.float32
BF = mybir.dt.bfloat16

NSPLIT = 4                # 512-wide n-chunks per (ct, t) matmul group


def build_nc():
    nc = bacc.Bacc("TRN2", target_bir_lowering=False, num_devices=NCORES)

    # ---- I/O (per-core tensors; e-chunk/c-shard baked into the data) ----
    xT    = nc.dram_tensor("xT",    [128, L], FP, kind="ExternalInput")      # x^T
    xrow  = nc.dram_tensor("xrow",  [CB, D], FP, kind="ExternalInput")       # own c-shard of x
    mT    = nc.dram_tensor("mT",    [128, T, L], BF, kind="ExternalInput")   # mT[p,t,c] = mask[c, t*128+p]
    mTc   = nc.dram_tensor("mTc",   [128, T, CB], BF, kind="ExternalInput")  # own-shard mask cols
    WlTc  = nc.dram_tensor("WlTc",  [128, EC], FP, kind="ExternalInput")     # Wl.T[:, e0:e0+EC]
    Wl2T  = nc.dram_tensor("Wl2T",  [128, 128], FP, kind="ExternalInput")    # Wl2.T
    WloBc = nc.dram_tensor("WloBc", [128, EC * 128], BF, kind="ExternalInput")  # Wlo[d, e0+j], bcast parts
    blcB  = nc.dram_tensor("blcB",  [128, EC], FP, kind="ExternalInput")     # bl[e0:e0+EC] bcast parts
    bl2B  = nc.dram_tensor("bl2B",  [128, 128], FP, kind="ExternalInput")    # bl2 bcast parts
    bloB  = nc.dram_tensor("bloB",  [CB, D], FP, kind="ExternalInput")       # blo bcast parts
    gam   = nc.dram_tensor("gam",   [CB, D], FP, kind="ExternalInput")
    bet   = nc.dram_tensor("bet",   [CB, D], FP, kind="ExternalInput")
    out   = nc.dram_tensor("out",   [CB, D], FP, kind="ExternalOutput")

    Sqrt = mybir.ActivationFunctionType.Sqrt
    mult = mybir.AluOpType.mult
    add = mybir.AluOpType.add
    bypass = mybir.AluOpType.bypass

    with tile.TileContext(nc) as tc:
        with (
            tc.tile_pool(name="singles", bufs=1) as singles,
            tc.tile_pool(name="mm", bufs=2, space="PSUM") as mmps,
            tc.tile_pool(name="v", bufs=2) as vpool,
            tc.tile_pool(name="tr", bufs=2) as trpool,
            tc.tile_pool(name="dram", bufs=1, space="DRAM") as dram,
        ):
            # ---- load inputs ----
            sb_xT = singles.tile([128, L], FP)
            nc.sync.dma_start(sb_xT, xT[:, :])
            sb_WlTc = singles.tile([128, EC], FP)
            nc.sync.dma_start(sb_WlTc, WlTc[:, :])
            sb_Wl2T = singles.tile([128, 128], FP)
            nc.sync.dma_start(sb_Wl2T, Wl2T[:, :])
            sb_WloBc = singles.tile([128, EC * 128], BF)
            nc.sync.dma_start(sb_WloBc, WloBc[:, :])
            sb_blcB = singles.tile([128, EC], FP)
            nc.sync.dma_start(sb_blcB, blcB[:, :])
            sb_bl2B = singles.tile([128, 128], FP)
            nc.sync.dma_start(sb_bl2B, bl2B[:, :])
            sb_mT = singles.tile([128, T, L], BF)
            nc.sync.dma_start(sb_mT, mT[:, :, :])
            sb_mTc = singles.tile([128, T, CB], BF)
            nc.sync.dma_start(sb_mTc, mTc[:, :, :])
            sb_bloB = singles.tile([CB, D], FP)
            nc.sync.dma_start(sb_bloB, bloB[:, :])
            sb_xrow = singles.tile([CB, D], FP)
            nc.sync.dma_start(sb_xrow, xrow[:, :])
            sb_gam = singles.tile([CB, D], FP)
            nc.sync.dma_start(sb_gam, gam[:, :])
            sb_bet = singles.tile([CB, D], FP)
            nc.sync.dma_start(sb_bet, bet[:, :])
            sb_eps = singles.tile([CB, 1], FP)
            nc.vector.memset(sb_eps, EPS)

            # ---- activations in natural layout (rows on partitions), bf16 ----
            # tx_nat[p, t, d] = tx[t*128+p, d];  a_sel[p, t, j] = a[t*128+p, e0+j]
            tx_nat = singles.tile([128, T, 128], BF)
            a_sel = singles.tile([128, T, EC], FP)    # fp32: ACT scale requirement
            for h in range(2):
                ps = mmps.tile([128, NSPLIT, 512], FP, tag="mm")
                for q in range(4):
                    t = h * 4 + q
                    sl = slice(t * 128, (t + 1) * 128)
                    nc.tensor.matmul(ps[:, q, 0:128], sb_xT[:, sl], sb_Wl2T,
                                     start=True, stop=True)
                    nc.tensor.matmul(ps[:, q, 128:128 + EC], sb_xT[:, sl], sb_WlTc,
                                     start=True, stop=True)
                for q in range(4):
                    t = h * 4 + q
                    nc.vector.tensor_add(tx_nat[:, t, :], ps[:, q, 0:128], sb_bl2B)
                    nc.vector.tensor_add(a_sel[:, t, :], ps[:, q, 128:128 + EC],
                                         sb_blcB)

            # ---- Z term for own shard: Z[c,d] = sum_a mask[c,a] tx[a,d] ----
            z_ps = mmps.tile([128, NSPLIT, 512], FP, tag="mm")
            for t in range(T):
                nc.tensor.matmul(z_ps[:, 0, 0:CB], sb_mTc[:, t, :], tx_nat[:, t, :],
                                 start=(t == 0), stop=(t == T - 1))
            # base = x_shard + blo * Z   (everything not needing the collective)
            base = singles.tile([CB, D], FP)
            nc.vector.scalar_tensor_tensor(base, z_ps[:, 0, 0:CB], 1.0, sb_bloB,
                                           op0=mult, op1=mult)
            nc.vector.tensor_add(base, base, sb_xrow)

            # ---- U'[a, j, d] = a[a,e0+j] * tx[a,d] * Wlo[d,e0+j]  (bf16) ----
            # two stages: atx = a*tx (tensor_scalar, 4x mode), then one big
            # tensor_tensor per a-tile against the flat Wlo broadcast (2x mode)
            atx = singles.tile([128, T, EC * 128], BF)
            up = singles.tile([128, T, EC * 128], BF)
            for t in range(T):
                for j in range(EC):
                    nc.vector.tensor_scalar_mul(
                        atx[:, t, j * 128:(j + 1) * 128],
                        tx_nat[:, t, :],
                        a_sel[:, t, j:j + 1],
                    )
                nc.vector.tensor_mul(up[:, t, :], atx[:, t, :], sb_WloBc)

            # ---- main loop over output c-tiles ----
            part_dram = dram.tile([L, D], BF)
            for ct in range(T):
                ps = mmps.tile([128, NSPLIT, 512], FP, tag="mm")
                for t in range(T):
                    lhsT = sb_mT[:, t, ct * 128:(ct + 1) * 128]
                    for n in range(NSPLIT):
                        nc.tensor.matmul(
                            ps[:, n, :], lhsT, up[:, t, n * 512:(n + 1) * 512],
                            start=(t == 0), stop=(t == T - 1),
                        )
                # V[c, j, d] = a[c, e0+j] * T'[c, j, d]   (ACT, PSUM->SBUF, bf16)
                v = vpool.tile([128, EC, 128], BF, tag="v")
                for j in range(EC):
                    nc.scalar.mul(v[:, j, :],
                                  ps[:, j // 4, (j % 4) * 128:(j % 4 + 1) * 128],
                                  a_sel[:, ct, j:j + 1])
                # tree-reduce over j -> partial[c, d]
                t8 = trpool.tile([128, 8, 128], BF, tag="t8")
                nc.vector.tensor_add(t8, v[:, 0:8, :], v[:, 8:16, :])
                t4 = trpool.tile([128, 4, 128], BF, tag="t4")
                nc.vector.tensor_add(t4, t8[:, 0:4, :], t8[:, 4:8, :])
                t2 = trpool.tile([128, 2, 128], BF, tag="t2")
                nc.vector.tensor_add(t2, t4[:, 0:2, :], t4[:, 2:4, :])
                p1 = trpool.tile([128, 128], BF, tag="p1")
                nc.vector.tensor_add(p1, t2[:, 0, :], t2[:, 1, :])
                nc.sync.dma_start(part_dram[ct * 128:(ct + 1) * 128, :], p1)

            # ---- cross-core exchange: AllToAll hands every core the 8
            # per-core partials of its own 128-row c-shard (pure data
            # movement in bf16); the e-sum then happens locally in fp32 ----
            a2a_dram = dram.tile([L, D], BF)
            nc.gpsimd.collective_compute(
                "AllToAll",
                bypass,
                replica_groups=[list(range(NCORES))],
                ins=[part_dram[:, :].opt()],
                outs=[a2a_dram[:, :].opt()],
            )
            red_sb = singles.tile([128, NCORES, D], BF)
            nc.sync.dma_start(red_sb, a2a_dram[:, :].rearrange("(s p) d -> p s d", p=128))
            r4 = singles.tile([128, 4, D], BF)
            nc.vector.tensor_add(r4, red_sb[:, 0:4, :], red_sb[:, 4:8, :])
            r2 = singles.tile([128, 2, D], BF)
            nc.vector.tensor_add(r2, r4[:, 0:2, :], r4[:, 2:4, :])
            y_sb = singles.tile([CB, D], FP)
            nc.vector.tensor_add(y_sb, r2[:, 0, :], r2[:, 1, :])

            # ---- y = base + reduced partials ; LayerNorm ----
            nc.vector.tensor_add(y_sb, y_sb, base)

            stats = singles.tile([CB, nc.vector.BN_STATS_DIM], FP)
            nc.vector.bn_stats(stats, y_sb)
            mv = singles.tile([CB, 2], FP)
            nc.vector.bn_aggr(mv, stats)
            nc.vector.tensor_scalar_sub(y_sb, y_sb, mv[:, 0:1])
            sd = singles.tile([CB, 1], FP)
            nc.scalar.activation(sd, mv[:, 1:2], Sqrt, bias=sb_eps, scale=1.0)
            rstd = singles.tile([CB, 1], FP)
            nc.vector.reciprocal(rstd, sd)
            nc.vector.tensor_scalar_mul(y_sb, y_sb, rstd)
            nc.vector.tensor_mul(y_sb, y_sb, sb_gam)
            nc.vector.tensor_add(y_sb, y_sb, sb_bet)

            nc.sync.dma_start(out[:, :], y_sb)

    return nc


_NC_CACHE = None


def _get_nc():
    global _NC_CACHE
    if _NC_CACHE is None:
        _NC_CACHE = build_nc()
        _NC_CACHE.finalize()
    return _NC_CACHE


def _prepare_in_maps(x, mask, Wl, bl, Wlo, blo, Wl2, bl2, gamma, beta):
    f32 = np.float32
    bf16 = ml_dtypes.bfloat16
    x0 = np.ascontiguousarray(np.asarray(x, f32)[0])          # [L, D]
    xT = np.ascontiguousarray(x0.T)                           # [128, L]
    m = np.asarray(mask)[0].astype(bf16)                      # [L(c), L(a)]
    # mT[p, t, c] = mask[c, t*128 + p]
    mT_full = np.ascontiguousarray(
        m.T.reshape(T, 128, L).transpose(1, 0, 2))            # [128, T, L]
    WlT = np.ascontiguousarray(np.asarray(Wl, f32).T)         # [in, e]
    Wl2T = np.ascontiguousarray(np.asarray(Wl2, f32).T)
    WloT = np.asarray(Wlo, f32).T                             # [e, d]
    bl_ = np.asarray(bl, f32)
    bl2B = np.ascontiguousarray(np.broadcast_to(np.asarray(bl2, f32), (128, 128)))
    bloB = np.ascontiguousarray(np.broadcast_to(np.asarray(blo, f32), (CB, D)))
    gam_b = np.ascontiguousarray(np.broadcast_to(np.asarray(gamma, f32), (CB, D)))
    bet_b = np.ascontiguousarray(np.broadcast_to(np.asarray(beta, f32), (CB, D)))

    in_maps = []
    for k in range(NCORES):
        blk = slice(k * CB, (k + 1) * CB)
        e0 = k * EC
        WloBc = np.ascontiguousarray(
            np.broadcast_to(WloT[e0:e0 + EC].astype(bf16).reshape(1, EC * 128),
                            (128, EC * 128)))
        in_maps.append({
            "xT": xT,
            "xrow": np.ascontiguousarray(x0[blk]),
            "mT": mT_full,
            "mTc": np.ascontiguousarray(mT_full[:, :, blk]),
            "WlTc": np.ascontiguousarray(WlT[:, e0:e0 + EC]),
            "Wl2T": Wl2T,
            "WloBc": WloBc,
            "blcB": np.ascontiguousarray(
                np.broadcast_to(bl_[e0:e0 + EC], (128, EC))),
            "bl2B": bl2B,
            "bloB": bloB,
            "gam": gam_b,
            "bet": bet_b,
        })
    return in_maps


def kernel(x, mask, Wl, bl, Wlo, blo, Wl2, bl2, gamma, beta):
    in_maps = _prepare_in_maps(x, mask, Wl, bl, Wlo, blo, Wl2, bl2, gamma, beta)
    res = run_bass_kernel_spmd(_get_nc(), in_maps, core_ids=list(range(NCORES)))
    y = np.concatenate([res.results[k]["out"] for k in range(NCORES)], axis=0)
    return y.reshape(B, L, D).astype(np.float32)
